# revision 49
# baseline (speedup 1.0000x reference)
"""GPT2 self-attention on 8 trn2 NeuronCores (tensor-parallel).

Sharding: core c handles batch b = c//4 and head-group g = c%4 (4 of 16
heads = 256 of 1024 dims).

Per core:
  1. Q/K projection: qkt [512 qk-dims, 2048 tokens] = w_qk^T @ x (x^T as rhs)
  2. V projection:   [2048 tokens, 256 v-dims] = x @ w_v (x^T tile as lhsT),
     stored per key-tile as [128, head, 65] with a ones column (col 64).
  3. Causal attention per head-pair, keys on PSUM partitions:
       S^T = K-tile.T @ Q-chunk (both heads into one 2-bank PSUM tile)
       -> diag mask matmul -> merged exp(S/8) on ACT -> probs bf16
       AV flipped: out[q-block 128, 65] += probs-block.T @ [V | 1]
       (col 64 = softmax denominator, landing per-query-partition)
     Normalize via DVE reciprocal + per-block tensor_scalar multiply.
  4. Transpose O_norm per 128-query block via DMA-transpose -> O^T [dims, q].
  5. Partial out-projection z^T_partial [1024, 2048] = w_out[own 256 rows]^T
     contribution, PSUM -> bf16 -> DRAM per [128, 512] tile (the output).

Host reorders/slices/casts inputs, and unshards by summing the four
tensor-parallel z^T partials per batch (f32) and transposing into
[B, S, D]. b_qkv/b_out are zeros by the problem spec and are folded out.
Matmuls run bf16 with fp32 PSUM accumulation.
"""

import numpy as np
import ml_dtypes
from contextlib import ExitStack

B, S, D, H = 2, 2048, 1024, 16
HD = 64            # head dim
NCORES = 8
HPC = 4            # heads per core
GD = HPC * HD      # 256 dims per core group
QW = 512           # query-chunk width
NEG = -1.0e9

_CACHE = {}


def _build_program():
    import concourse.tile as tile
    from concourse import bacc, mybir

    bf16 = mybir.dt.bfloat16
    f32 = mybir.dt.float32

    nc = bacc.Bacc("TRN2", target_bir_lowering=False, debug=False,
                   num_devices=NCORES)

    xt = nc.dram_tensor("xt", [D, S], bf16, kind="ExternalInput").ap()
    wqk = nc.dram_tensor("wqk", [D, 2 * GD], bf16, kind="ExternalInput").ap()
    wv = nc.dram_tensor("wv", [D, GD], bf16, kind="ExternalInput").ap()
    wo = nc.dram_tensor("wo", [GD, D], bf16, kind="ExternalInput").ap()
    mtril = nc.dram_tensor("mtril", [128, 128], bf16, kind="ExternalInput").ap()
    ident = nc.dram_tensor("ident", [128, 128], bf16, kind="ExternalInput").ap()
    ztp = nc.dram_tensor("ztp", [D, S], bf16, kind="ExternalOutput").ap()

    NKT = S // 128          # 16 key tiles
    KD = D // 128           # 8 contraction tiles over d_model
    NQC = S // QW           # 4 query chunks

    with tile.TileContext(nc) as tc, ExitStack() as ctx:
        persist = ctx.enter_context(tc.tile_pool(name="persist", bufs=1))
        # PSUM budget (8 banks): pscore 2x2 + pot 1x2 + pmisc 2x1 = 8
        pscore = ctx.enter_context(tc.tile_pool(name="pscore", bufs=2, space="PSUM"))
        pot = ctx.enter_context(tc.tile_pool(name="pot", bufs=1, space="PSUM"))
        pmisc = ctx.enter_context(tc.tile_pool(name="pmisc", bufs=2, space="PSUM"))
        prpool = ctx.enter_context(tc.tile_pool(name="prpool", bufs=34))
        onpool = ctx.enter_context(tc.tile_pool(name="onpool", bufs=3))
        ottpool = ctx.enter_context(tc.tile_pool(name="ottpool", bufs=8))
        recpool = ctx.enter_context(tc.tile_pool(name="recpool", bufs=3))
        zsbpool = ctx.enter_context(tc.tile_pool(name="zsbpool", bufs=3))
        dram_pool = ctx.enter_context(tc.tile_pool(name="dram_pool", bufs=1, space="DRAM"))

        xt_sb = [persist.tile([128, S], bf16, tag=f"xt{k}", name=f"xt{k}") for k in range(KD)]
        wqk_sb = [persist.tile([128, 2 * GD], bf16, tag=f"wqk{k}", name=f"wqk{k}") for k in range(KD)]
        wv_sb = [persist.tile([128, GD], bf16, tag=f"wv{k}", name=f"wv{k}") for k in range(KD)]
        wo_sb = [persist.tile([128, D], bf16, tag=f"wo{j}", name=f"wo{j}") for j in range(2)]
        mtril_sb = persist.tile([128, 128], bf16, tag="mtril", name="mtril_sb")
        ident_sb = persist.tile([128, 128], bf16, tag="ident", name="ident_sb")
        qkt_sb = [persist.tile([128, S], bf16, tag=f"qkt{m}", name=f"qkt{m}") for m in range(4)]
        v_sb = [persist.tile([128, HPC, HD + 1], bf16, tag=f"v{t}", name=f"v{t}") for t in range(NKT)]



        # ---- input loads: attention-critical columns first ----
        nc.gpsimd.dma_start(out=mtril_sb[:], in_=mtril[:])
        nc.gpsimd.dma_start(out=ident_sb[:], in_=ident[:])
        for k in range(KD):
            nc.sync.dma_start(out=xt_sb[k][:, 0:QW], in_=xt[k * 128:(k + 1) * 128, 0:QW])
            nc.scalar.dma_start(out=wqk_sb[k][:], in_=wqk[k * 128:(k + 1) * 128, :])
        for k in range(KD):
            nc.gpsimd.dma_start(out=wv_sb[k][:], in_=wv[k * 128:(k + 1) * 128, :])
        for j in range(2):
            nc.gpsimd.dma_start(out=wo_sb[j][:], in_=wo[j * 128:(j + 1) * 128, :])
        for n in range(1, NQC):
            for k in range(KD):
                eng = nc.sync if k % 2 == 0 else nc.scalar
                eng.dma_start(out=xt_sb[k][:, n * QW:(n + 1) * QW],
                              in_=xt[k * 128:(k + 1) * 128, n * QW:(n + 1) * QW])

        # ---- projection helpers (PE fill work) ----
        def qkt_chunk(m, n):
            ps = pmisc.tile([128, QW], f32, tag="misc", name="qkt_ps")
            for k in range(KD):
                nc.tensor.matmul(
                    ps[:],
                    wqk_sb[k][:, m * 128:(m + 1) * 128],
                    xt_sb[k][:, n * QW:(n + 1) * QW],
                    start=(k == 0), stop=(k == KD - 1),
                )
            nc.vector.tensor_copy(qkt_sb[m][:, n * QW:(n + 1) * QW], ps[:])

        def v_tile(t):
            ps = pmisc.tile([128, GD], f32, tag="misc", name="v_ps")
            for k in range(KD):
                nc.tensor.matmul(
                    ps[:, 0:GD],
                    xt_sb[k][:, t * 128:(t + 1) * 128],
                    wv_sb[k][:],
                    start=(k == 0), stop=(k == KD - 1),
                )
            nc.vector.tensor_copy(
                v_sb[t][:, :, 0:HD],
                ps[:, 0:GD].rearrange("p (h d) -> p h d", h=HPC),
            )
            nc.vector.memset(v_sb[t][:, :, HD:HD + 1], 1.0)

        ott_of = {}

        def zp_step(qc, ct, epilogue=False, half=None):
            """One out-proj column tile: z^T[ct*128:+128, qc*512:+512]."""
            pool = pot if (epilogue and ct % 3 == 1) else pmisc
            w = QW if half is None else QW // 2
            qlo = qc * QW + (0 if not half else QW // 2)
            ps = pool.tile([128, w], f32, tag="misc" if pool is pmisc else "ot",
                           name="zp_ps")
            for pair in (0, 1):
                src = ott_of[(pair, qc)][:]
                if half is not None:
                    src = ott_of[(pair, qc)][:, half * (QW // 2):(half + 1) * (QW // 2)]
                nc.tensor.matmul(
                    ps[:],
                    wo_sb[pair][:, ct * 128:(ct + 1) * 128],
                    src,
                    start=(pair == 0), stop=(pair == 1),
                )
            zsb = zsbpool.tile([128, w], bf16, tag="zsb", name="zsb")
            if epilogue and ct % 2 == 1:
                nc.scalar.activation(zsb[:], ps[:],
                                     mybir.ActivationFunctionType.Copy)
            else:
                nc.vector.tensor_copy(zsb[:], ps[:])
            if epilogue:
                eng = nc.sync if ct % 2 == 0 else nc.scalar
            else:
                eng = nc.gpsimd
            eng.dma_start(
                out=ztp[ct * 128:(ct + 1) * 128, qlo:qlo + w],
                in_=zsb[:])

        # ---- attention ----
        def emit_scores(pair, qc, kt, prs):
            qstart = qc * QW
            j = kt - 4 * qc
            qoff = max(0, 128 * j)
            sp = pscore.tile([128, 1024], f32, tag="sc", name="sc_ps")
            pr = prpool.tile([128, 1024], bf16, tag="pr", name="pr_sb")
            for hh in range(2):
                base = 64 * hh
                nc.tensor.matmul(
                    sp[:, 512 * hh + qoff:512 * hh + 512],
                    qkt_sb[2 + pair][base:base + 64, kt * 128:(kt + 1) * 128],
                    qkt_sb[pair][base:base + 64, qstart + qoff:qstart + QW],
                    start=True, stop=True,
                )
            sp3 = sp[:].rearrange("p (h q) -> p h q", h=2)
            pr3 = pr[:].rearrange("p (h q) -> p h q", h=2)
            nc.scalar.activation(
                pr3[:, :, qoff:QW], sp3[:, :, qoff:QW],
                mybir.ActivationFunctionType.Exp,
                scale=0.125,
            )
            if j >= 0:
                # causal mask: zero future-key probs in the diagonal tile
                nc.vector.tensor_tensor(
                    out=pr3[:, :, qoff:qoff + 128],
                    in0=pr3[:, :, qoff:qoff + 128],
                    in1=mtril_sb[:].rearrange("p (o c) -> p o c", o=1).to_broadcast([128, 2, 128]),
                    op=mybir.AluOpType.mult,
                )
            prs[kt] = pr

        def av_block(pair, qc, ot, prs, hh, qb):
            """One (head, query-block) AV accumulation group: consecutive
            matmuls over its key tiles (one open PSUM group per bank)."""
            blk = hh * 4 + qb
            last = 4 * qc + qb
            for kt in range(last + 1):
                pr3 = prs[kt][:].rearrange("p (h q) -> p h q", h=2)
                nc.tensor.matmul(
                    ot[:, 128 * blk:128 * blk + HD + 1],
                    pr3[:, hh, qb * 128:(qb + 1) * 128],
                    v_sb[kt][:, 2 * pair + hh, :],
                    start=(kt == 0), stop=(kt == last),
                )

        def norm_transpose(pair, qc, ot, pe_transpose=False):
            ot3 = ot[:].rearrange("p (b q) -> p b q", b=8)
            rec = recpool.tile([128, 8], f32, tag="rec", name="rec_sb")
            nc.vector.reciprocal(rec[:], ot3[:, :, HD:HD + 1])
            onorm = onpool.tile([128, QW], bf16, tag="on", name="on_sb")
            nc.vector.tensor_tensor(
                out=onorm[:].rearrange("p (qb hh d) -> p hh qb d", qb=4, hh=2),
                in0=ot[:].rearrange("p (hh qb c) -> p hh qb c", hh=2, qb=4)[:, :, :, 0:HD],
                in1=rec[:].rearrange("p (hh qb) -> p hh qb", hh=2).to_broadcast([128, 2, 4, HD]),
                op=mybir.AluOpType.mult,
            )
            ott = ottpool.tile([128, QW], bf16, tag=f"ott{pair}", name="ott_sb")
            if pe_transpose:
                # tail-critical: PE is idle here and skips the DMA-queue latency
                tp = pmisc.tile([128, QW], bf16, tag="misc", name="tp_ps")
                for qb in range(4):
                    nc.tensor.transpose(
                        tp[:, qb * 128:(qb + 1) * 128],
                        onorm[:, qb * 128:(qb + 1) * 128],
                        ident_sb[:])
                nc.vector.tensor_copy(ott[:], tp[:])
            else:
                for qb in range(4):
                    nc.sync.dma_start_transpose(
                        ott[:, qb * 128:(qb + 1) * 128],
                        onorm[:, qb * 128:(qb + 1) * 128])
            ott_of[(pair, qc)] = ott

        # ---- main schedule ----
        qkt_chunk(0, 0)
        qkt_chunk(2, 0)
        for t in range(4):
            v_tile(t)
        qkt_chunk(1, 0)
        qkt_chunk(3, 0)

        for qc in range(NQC):
            nkt = 4 * qc + 4
            fills = []
            # v tiles for THIS round's AV phase: consumed during pair0 scores
            if qc >= 1:
                for t in range(4 * qc, 4 * qc + 4):
                    fills.append(lambda t=t: v_tile(t))
            if qc < NQC - 1:
                fills.append(lambda n=qc + 1: qkt_chunk(0, n))
                fills.append(lambda n=qc + 1: qkt_chunk(2, n))
            if qc == NQC - 1:
                # pair1's Q/K for this round: ready before the pair1 phase
                fills.append(lambda n=qc: qkt_chunk(1, n))
                fills.append(lambda n=qc: qkt_chunk(3, n))
            elif qc < NQC - 2:
                fills.append(lambda n=qc + 1: qkt_chunk(1, n))
                fills.append(lambda n=qc + 1: qkt_chunk(3, n))
            # all zp waves deferred to round 3 — the only PE-starved round
            if qc == NQC - 1:
                for q in range(NQC - 1):
                    for ct in range(KD):
                        fills.append(lambda c=ct, q=q: zp_step(q, c))

            prs0, prs1 = {}, {}
            # pair0 scores (ACT pipeline starts) with projection fills
            for kt in range(nkt):
                emit_scores(0, qc, kt, prs0)
                if fills:
                    fills.pop(0)()
            # pair1 scores keep ACT busy; pair0 AV blocks + fills cover PE
            ot0 = pot.tile([128, 1024], f32, tag="ot", name="ot_ps")
            avq = [(hh, qb) for hh in range(2) for qb in range(4)]
            for kt in range(nkt):
                emit_scores(1, qc, kt, prs1)
                for _ in range(2 if nkt <= 4 else 1):
                    if avq:
                        hh, qb = avq.pop(0)
                        av_block(0, qc, ot0, prs0, hh, qb)
                if fills:
                    fills.pop(0)()
            while avq:
                hh, qb = avq.pop(0)
                av_block(0, qc, ot0, prs0, hh, qb)
            norm_transpose(0, qc, ot0)
            ot1 = pot.tile([128, 1024], f32, tag="ot", name="ot_ps")
            if qc < NQC - 1:
                # pair1 AV blocks with remaining fills
                for hh in range(2):
                    for qb in range(4):
                        av_block(1, qc, ot1, prs1, hh, qb)
                        if fills:
                            fills.pop(0)()
                norm_transpose(1, qc, ot1)
                for f in fills:
                    f()
            else:
                # ---- final round, pair1: per-half pipelined epilogue ----
                ot13 = ot1[:].rearrange("p (hh qb c) -> p hh qb c", hh=2, qb=4)
                onorm = onpool.tile([128, QW], bf16, tag="on", name="on_sb")
                tpA = pscore.tile([128, 256], bf16, tag="sc", name="tpA_ps")
                tpB = pscore.tile([128, 256], bf16, tag="sc", name="tpB_ps")
                ott = ottpool.tile([128, QW], bf16, tag="ott1", name="ott_sb")
                ott_of[(1, qc)] = ott

                def avpair(qb):
                    av_block(1, qc, ot1, prs1, 0, qb)
                    av_block(1, qc, ot1, prs1, 1, qb)

                def normqb(qb):
                    rq = recpool.tile([128, 2], f32, tag="rec", name="rq_sb")
                    nc.vector.reciprocal(
                        rq[:], ot13[:, :, qb:qb + 1, HD:HD + 1])
                    nc.vector.tensor_tensor(
                        out=onorm[:, qb * 128:(qb + 1) * 128].rearrange(
                            "p (hh o d) -> p hh o d", hh=2, o=1),
                        in0=ot13[:, :, qb:qb + 1, 0:HD],
                        in1=rq[:].rearrange("p (hh o) -> p hh o", hh=2)
                            .to_broadcast([128, 2, 1, HD]),
                        op=mybir.AluOpType.mult,
                    )

                avpair(0)
                if fills:
                    fills.pop(0)()
                normqb(0)
                avpair(1)
                if fills:
                    fills.pop(0)()
                normqb(1)
                avpair(2)
                for qb in (0, 1):
                    nc.tensor.transpose(
                        tpA[:, (qb % 2) * 128:(qb % 2) * 128 + 128],
                        onorm[:, qb * 128:(qb + 1) * 128], ident_sb[:])
                normqb(2)
                nc.vector.tensor_copy(ott[:, 0:256], tpA[:])
                avpair(3)
                normqb(3)
                for f in fills:
                    f()
                for ct in range(KD):
                    zp_step(qc, ct, epilogue=True, half=0)
                for qb in (2, 3):
                    nc.tensor.transpose(
                        tpB[:, (qb % 2) * 128:(qb % 2) * 128 + 128],
                        onorm[:, qb * 128:(qb + 1) * 128], ident_sb[:])
                nc.vector.tensor_copy(ott[:, 256:QW], tpB[:])
                for ct in range(KD):
                    zp_step(qc, ct, epilogue=True, half=1)

    nc.compile()
    return nc


def _get_program():
    if "nc" not in _CACHE:
        _CACHE["nc"] = _build_program()
    return _CACHE["nc"]


def _make_in_maps(x, w_qkv, w_out):
    bf = ml_dtypes.bfloat16
    # probs layout [key, query]: keep q >= k (upper triangle incl diagonal)
    mtril = np.triu(np.ones((128, 128), dtype=np.float32), 0).astype(bf)
    ident = np.eye(128, dtype=np.float32).astype(bf)
    in_maps = []
    for c in range(NCORES):
        b, g = c // 4, c % 4
        cs = slice(GD * g, GD * (g + 1))
        xt = np.ascontiguousarray(x[b].T).astype(bf)
        wqk = np.concatenate(
            [w_qkv[:, cs], w_qkv[:, D + GD * g:D + GD * (g + 1)]], axis=1
        ).astype(bf)
        wv = np.ascontiguousarray(w_qkv[:, 2 * D + GD * g:2 * D + GD * (g + 1)]).astype(bf)
        wo = np.ascontiguousarray(w_out[cs, :]).astype(bf)
        in_maps.append(
            {"xt": xt, "wqk": wqk, "wv": wv, "wo": wo, "mtril": mtril,
             "ident": ident})
    return in_maps


def kernel(x, w_qkv, b_qkv, w_out, b_out):
    from concourse.bass_utils import run_bass_kernel_spmd

    x = np.asarray(x, dtype=np.float32)
    w_qkv = np.asarray(w_qkv, dtype=np.float32)
    w_out = np.asarray(w_out, dtype=np.float32)

    nc = _get_program()
    in_maps = _make_in_maps(x, w_qkv, w_out)
    res = run_bass_kernel_spmd(nc, in_maps, list(range(NCORES))).results

    # unshard: sum the 4 TP partial z^T contributions per batch, transpose
    out = np.empty((B, S, D), dtype=np.float32)
    for b in range(B):
        acc = np.zeros((D, S), dtype=np.float32)
        for g in range(4):
            acc += res[4 * b + g]["ztp"].astype(np.float32)
        out[b] = acc.T
    return out


# revision 51
# speedup vs baseline: 1.0388x; 1.0388x over previous
"""GPT2 self-attention on 8 trn2 NeuronCores (tensor-parallel).

Sharding: core c handles batch b = c//4 and head-group g = c%4 (4 of 16
heads = 256 of 1024 dims).

Per core:
  1. Q/K projection: qkt [512 qk-dims, 2048 tokens] = w_qk^T @ x (x^T as rhs)
  2. V projection:   [2048 tokens, 256 v-dims] = x @ w_v (x^T tile as lhsT),
     stored per key-tile as [128, head, 65] with a ones column (col 64).
  3. Causal attention per head-pair, keys on PSUM partitions:
       S^T = K-tile.T @ Q-chunk (both heads into one 2-bank PSUM tile)
       -> diag mask matmul -> merged exp(S/8) on ACT -> probs bf16
       AV flipped: out[q-block 128, 65] += probs-block.T @ [V | 1]
       (col 64 = softmax denominator, landing per-query-partition)
     Normalize via DVE reciprocal + per-block tensor_scalar multiply.
  4. Transpose O_norm per 128-query block via DMA-transpose -> O^T [dims, q].
  5. Partial out-projection z^T_partial [1024, 2048] = w_out[own 256 rows]^T
     contribution, PSUM -> bf16 -> DRAM per [128, 512] tile (the output).

Host reorders/slices/casts inputs, and unshards by summing the four
tensor-parallel z^T partials per batch (f32) and transposing into
[B, S, D]. b_qkv/b_out are zeros by the problem spec and are folded out.
Matmuls run bf16 with fp32 PSUM accumulation.
"""

import numpy as np
import ml_dtypes
from contextlib import ExitStack

B, S, D, H = 2, 2048, 1024, 16
HD = 64            # head dim
NCORES = 8
HPC = 4            # heads per core
GD = HPC * HD      # 256 dims per core group
QW = 512           # query-chunk width
NEG = -1.0e9

_CACHE = {}


def _build_program():
    import concourse.tile as tile
    from concourse import bacc, mybir

    bf16 = mybir.dt.bfloat16
    f32 = mybir.dt.float32

    nc = bacc.Bacc("TRN2", target_bir_lowering=False, debug=False,
                   num_devices=NCORES)

    xt = nc.dram_tensor("xt", [D, S], bf16, kind="ExternalInput").ap()
    wqk = nc.dram_tensor("wqk", [D, 2 * GD], bf16, kind="ExternalInput").ap()
    wv = nc.dram_tensor("wv", [D, GD], bf16, kind="ExternalInput").ap()
    wo = nc.dram_tensor("wo", [GD, D], bf16, kind="ExternalInput").ap()
    mtril = nc.dram_tensor("mtril", [128, 128], bf16, kind="ExternalInput").ap()
    ident = nc.dram_tensor("ident", [128, 128], bf16, kind="ExternalInput").ap()
    ztp = nc.dram_tensor("ztp", [D, S], bf16, kind="ExternalOutput").ap()

    NKT = S // 128          # 16 key tiles
    KD = D // 128           # 8 contraction tiles over d_model
    NQC = S // QW           # 4 query chunks

    with tile.TileContext(nc) as tc, ExitStack() as ctx:
        persist = ctx.enter_context(tc.tile_pool(name="persist", bufs=1))
        # PSUM budget (8 banks): pscore 2x2 + pot 1x2 + pmisc 2x1 = 8
        pscore = ctx.enter_context(tc.tile_pool(name="pscore", bufs=2, space="PSUM"))
        pot = ctx.enter_context(tc.tile_pool(name="pot", bufs=1, space="PSUM"))
        pmisc = ctx.enter_context(tc.tile_pool(name="pmisc", bufs=2, space="PSUM"))
        prpool = ctx.enter_context(tc.tile_pool(name="prpool", bufs=34))
        onpool = ctx.enter_context(tc.tile_pool(name="onpool", bufs=3))
        ottpool = ctx.enter_context(tc.tile_pool(name="ottpool", bufs=8))
        recpool = ctx.enter_context(tc.tile_pool(name="recpool", bufs=3))
        zsbpool = ctx.enter_context(tc.tile_pool(name="zsbpool", bufs=3))
        dram_pool = ctx.enter_context(tc.tile_pool(name="dram_pool", bufs=1, space="DRAM"))

        xt_sb = [persist.tile([128, S], bf16, tag=f"xt{k}", name=f"xt{k}") for k in range(KD)]
        wqk_sb = [persist.tile([128, 2 * GD], bf16, tag=f"wqk{k}", name=f"wqk{k}") for k in range(KD)]
        wv_sb = [persist.tile([128, GD], bf16, tag=f"wv{k}", name=f"wv{k}") for k in range(KD)]
        wo_sb = [persist.tile([128, D], bf16, tag=f"wo{j}", name=f"wo{j}") for j in range(2)]
        mtril_sb = persist.tile([128, 128], bf16, tag="mtril", name="mtril_sb")
        ident_sb = persist.tile([128, 128], bf16, tag="ident", name="ident_sb")
        qkt_sb = [persist.tile([128, S], bf16, tag=f"qkt{m}", name=f"qkt{m}") for m in range(4)]
        v_sb = [persist.tile([128, HPC, HD + 1], bf16, tag=f"v{t}", name=f"v{t}") for t in range(NKT)]



        # ---- input loads: attention-critical columns first ----
        nc.gpsimd.dma_start(out=mtril_sb[:], in_=mtril[:])
        nc.gpsimd.dma_start(out=ident_sb[:], in_=ident[:])
        for k in range(KD):
            nc.sync.dma_start(out=xt_sb[k][:, 0:QW], in_=xt[k * 128:(k + 1) * 128, 0:QW])
            nc.scalar.dma_start(out=wqk_sb[k][:], in_=wqk[k * 128:(k + 1) * 128, :])
        for k in range(KD):
            nc.gpsimd.dma_start(out=wv_sb[k][:], in_=wv[k * 128:(k + 1) * 128, :])
        for j in range(2):
            nc.gpsimd.dma_start(out=wo_sb[j][:], in_=wo[j * 128:(j + 1) * 128, :])
        for n in range(1, NQC):
            for k in range(KD):
                eng = nc.sync if k % 2 == 0 else nc.scalar
                eng.dma_start(out=xt_sb[k][:, n * QW:(n + 1) * QW],
                              in_=xt[k * 128:(k + 1) * 128, n * QW:(n + 1) * QW])

        # ---- projection helpers (PE fill work) ----
        def qkt_chunk(m, n):
            ps = pmisc.tile([128, QW], f32, tag="misc", name="qkt_ps")
            for k in range(KD):
                nc.tensor.matmul(
                    ps[:],
                    wqk_sb[k][:, m * 128:(m + 1) * 128],
                    xt_sb[k][:, n * QW:(n + 1) * QW],
                    start=(k == 0), stop=(k == KD - 1),
                )
            nc.vector.tensor_copy(qkt_sb[m][:, n * QW:(n + 1) * QW], ps[:])

        def v_tile(t):
            ps = pmisc.tile([128, GD], f32, tag="misc", name="v_ps")
            for k in range(KD):
                nc.tensor.matmul(
                    ps[:, 0:GD],
                    xt_sb[k][:, t * 128:(t + 1) * 128],
                    wv_sb[k][:],
                    start=(k == 0), stop=(k == KD - 1),
                )
            nc.vector.tensor_copy(
                v_sb[t][:, :, 0:HD],
                ps[:, 0:GD].rearrange("p (h d) -> p h d", h=HPC),
            )
            nc.vector.memset(v_sb[t][:, :, HD:HD + 1], 1.0)

        ott_of = {}

        def zp_step(qc, ct, epilogue=False):
            """One out-proj column tile: z^T[ct*128:+128, qc*512:+512]."""
            pool = pscore if (epilogue and ct % 2 == 1) else pmisc
            ps = pool.tile([128, QW], f32, tag="misc" if pool is pmisc else "sc",
                           name="zp_ps")
            for pair in (0, 1):
                nc.tensor.matmul(
                    ps[:],
                    wo_sb[pair][:, ct * 128:(ct + 1) * 128],
                    ott_of[(pair, qc)][:],
                    start=(pair == 0), stop=(pair == 1),
                )
            zsb = zsbpool.tile([128, QW], bf16, tag="zsb", name="zsb")
            if epilogue and ct % 2 == 1:
                nc.scalar.activation(zsb[:], ps[:],
                                     mybir.ActivationFunctionType.Copy)
            else:
                nc.vector.tensor_copy(zsb[:], ps[:])
            if epilogue:
                eng = nc.sync if ct % 2 == 0 else nc.scalar
            else:
                eng = nc.gpsimd
            eng.dma_start(
                out=ztp[ct * 128:(ct + 1) * 128, qc * QW:(qc + 1) * QW],
                in_=zsb[:])

        # ---- attention ----
        def emit_scores(pair, qc, kt, prs):
            qstart = qc * QW
            j = kt - 4 * qc
            qoff = max(0, 128 * j)
            sp = pscore.tile([128, 1024], f32, tag="sc", name="sc_ps")
            pr = prpool.tile([128, 1024], bf16, tag="pr", name="pr_sb")
            for hh in range(2):
                base = 64 * hh
                nc.tensor.matmul(
                    sp[:, 512 * hh + qoff:512 * hh + 512],
                    qkt_sb[2 + pair][base:base + 64, kt * 128:(kt + 1) * 128],
                    qkt_sb[pair][base:base + 64, qstart + qoff:qstart + QW],
                    start=True, stop=True,
                )
            sp3 = sp[:].rearrange("p (h q) -> p h q", h=2)
            pr3 = pr[:].rearrange("p (h q) -> p h q", h=2)
            nc.scalar.activation(
                pr3[:, :, qoff:QW], sp3[:, :, qoff:QW],
                mybir.ActivationFunctionType.Exp,
                scale=0.125,
            )
            if j >= 0:
                # causal mask: zero future-key probs in the diagonal tile
                nc.vector.tensor_tensor(
                    out=pr3[:, :, qoff:qoff + 128],
                    in0=pr3[:, :, qoff:qoff + 128],
                    in1=mtril_sb[:].rearrange("p (o c) -> p o c", o=1).to_broadcast([128, 2, 128]),
                    op=mybir.AluOpType.mult,
                )
            prs[kt] = pr

        def av_block(pair, qc, ot, prs, hh, qb):
            """One (head, query-block) AV accumulation group: consecutive
            matmuls over its key tiles (one open PSUM group per bank)."""
            blk = hh * 4 + qb
            last = 4 * qc + qb
            for kt in range(last + 1):
                pr3 = prs[kt][:].rearrange("p (h q) -> p h q", h=2)
                nc.tensor.matmul(
                    ot[:, 128 * blk:128 * blk + HD + 1],
                    pr3[:, hh, qb * 128:(qb + 1) * 128],
                    v_sb[kt][:, 2 * pair + hh, :],
                    start=(kt == 0), stop=(kt == last),
                )

        def norm_transpose(pair, qc, ot, pe_transpose=False):
            ot3 = ot[:].rearrange("p (b q) -> p b q", b=8)
            rec = recpool.tile([128, 8], f32, tag="rec", name="rec_sb")
            nc.vector.reciprocal(rec[:], ot3[:, :, HD:HD + 1])
            onorm = onpool.tile([128, QW], bf16, tag="on", name="on_sb")
            nc.vector.tensor_tensor(
                out=onorm[:].rearrange("p (qb hh d) -> p hh qb d", qb=4, hh=2),
                in0=ot[:].rearrange("p (hh qb c) -> p hh qb c", hh=2, qb=4)[:, :, :, 0:HD],
                in1=rec[:].rearrange("p (hh qb) -> p hh qb", hh=2).to_broadcast([128, 2, 4, HD]),
                op=mybir.AluOpType.mult,
            )
            ott = ottpool.tile([128, QW], bf16, tag=f"ott{pair}", name="ott_sb")
            if pe_transpose:
                # tail-critical: PE is idle here and skips the DMA-queue latency
                tp = pmisc.tile([128, QW], bf16, tag="misc", name="tp_ps")
                for qb in range(4):
                    nc.tensor.transpose(
                        tp[:, qb * 128:(qb + 1) * 128],
                        onorm[:, qb * 128:(qb + 1) * 128],
                        ident_sb[:])
                nc.vector.tensor_copy(ott[:], tp[:])
            else:
                for qb in range(4):
                    nc.sync.dma_start_transpose(
                        ott[:, qb * 128:(qb + 1) * 128],
                        onorm[:, qb * 128:(qb + 1) * 128])
            ott_of[(pair, qc)] = ott

        # ---- main schedule ----
        qkt_chunk(0, 0)
        qkt_chunk(2, 0)
        for t in range(4):
            v_tile(t)
        qkt_chunk(1, 0)
        qkt_chunk(3, 0)

        for qc in range(NQC):
            nkt = 4 * qc + 4
            fills = []
            # v tiles for THIS round's AV phase: consumed during pair0 scores
            if qc >= 1:
                for t in range(4 * qc, 4 * qc + 4):
                    fills.append(lambda t=t: v_tile(t))
            if qc < NQC - 1:
                fills.append(lambda n=qc + 1: qkt_chunk(0, n))
                fills.append(lambda n=qc + 1: qkt_chunk(2, n))
            if qc == NQC - 1:
                # pair1's Q/K for this round: ready before the pair1 phase
                fills.append(lambda n=qc: qkt_chunk(1, n))
                fills.append(lambda n=qc: qkt_chunk(3, n))
            elif qc < NQC - 2:
                fills.append(lambda n=qc + 1: qkt_chunk(1, n))
                fills.append(lambda n=qc + 1: qkt_chunk(3, n))
            # all zp waves deferred to round 3 — the only PE-starved round
            if qc == NQC - 1:
                for q in range(NQC - 1):
                    for ct in range(KD):
                        fills.append(lambda c=ct, q=q: zp_step(q, c))

            prs0, prs1 = {}, {}
            # pair0 scores (ACT pipeline starts) with projection fills
            for kt in range(nkt):
                emit_scores(0, qc, kt, prs0)
                if fills:
                    fills.pop(0)()
            # pair1 scores keep ACT busy; pair0 AV blocks + fills cover PE
            ot0 = pot.tile([128, 1024], f32, tag="ot", name="ot_ps")
            avq = [(hh, qb) for hh in range(2) for qb in range(4)]
            for kt in range(nkt):
                emit_scores(1, qc, kt, prs1)
                for _ in range(2 if nkt <= 4 else 1):
                    if avq:
                        hh, qb = avq.pop(0)
                        av_block(0, qc, ot0, prs0, hh, qb)
                if fills:
                    fills.pop(0)()
            while avq:
                hh, qb = avq.pop(0)
                av_block(0, qc, ot0, prs0, hh, qb)
            norm_transpose(0, qc, ot0)
            # pair1 AV blocks with remaining fills
            ot1 = pot.tile([128, 1024], f32, tag="ot", name="ot_ps")
            if qc < NQC - 1:
                for hh in range(2):
                    for qb in range(4):
                        av_block(1, qc, ot1, prs1, hh, qb)
                        if fills:
                            fills.pop(0)()
                norm_transpose(1, qc, ot1)
            else:
                # final round: qb-major AV with per-qb norms hidden under
                # the next blocks' AV matmuls, then PE transposes + copy
                ot13 = ot1[:].rearrange("p (hh qb c) -> p hh qb c", hh=2, qb=4)
                onorm = onpool.tile([128, QW], bf16, tag="on", name="on_sb")

                def normqb(qb):
                    rq = recpool.tile([128, 2], f32, tag="rec", name="rq_sb")
                    nc.vector.reciprocal(
                        rq[:], ot13[:, :, qb:qb + 1, HD:HD + 1])
                    nc.vector.tensor_tensor(
                        out=onorm[:, qb * 128:(qb + 1) * 128].rearrange(
                            "p (hh o d) -> p hh o d", hh=2, o=1),
                        in0=ot13[:, :, qb:qb + 1, 0:HD],
                        in1=rq[:].rearrange("p (hh o) -> p hh o", hh=2)
                            .to_broadcast([128, 2, 1, HD]),
                        op=mybir.AluOpType.mult,
                    )

                for qb in range(4):
                    av_block(1, qc, ot1, prs1, 0, qb)
                    av_block(1, qc, ot1, prs1, 1, qb)
                    if fills:
                        fills.pop(0)()
                    normqb(qb)
                tp = pmisc.tile([128, QW], bf16, tag="misc", name="tp_ps")
                for qb in range(4):
                    nc.tensor.transpose(
                        tp[:, qb * 128:(qb + 1) * 128],
                        onorm[:, qb * 128:(qb + 1) * 128], ident_sb[:])
                ott = ottpool.tile([128, QW], bf16, tag="ott1", name="ott_sb")
                nc.vector.tensor_copy(ott[:], tp[:])
                ott_of[(1, qc)] = ott
            for f in fills:
                f()

        for ct in range(KD):
            zp_step(NQC - 1, ct, epilogue=True)

    nc.compile()
    return nc


def _get_program():
    if "nc" not in _CACHE:
        _CACHE["nc"] = _build_program()
    return _CACHE["nc"]


def _make_in_maps(x, w_qkv, w_out):
    bf = ml_dtypes.bfloat16
    # probs layout [key, query]: keep q >= k (upper triangle incl diagonal)
    mtril = np.triu(np.ones((128, 128), dtype=np.float32), 0).astype(bf)
    ident = np.eye(128, dtype=np.float32).astype(bf)
    in_maps = []
    for c in range(NCORES):
        b, g = c // 4, c % 4
        cs = slice(GD * g, GD * (g + 1))
        xt = np.ascontiguousarray(x[b].T).astype(bf)
        wqk = np.concatenate(
            [w_qkv[:, cs], w_qkv[:, D + GD * g:D + GD * (g + 1)]], axis=1
        ).astype(bf)
        wv = np.ascontiguousarray(w_qkv[:, 2 * D + GD * g:2 * D + GD * (g + 1)]).astype(bf)
        wo = np.ascontiguousarray(w_out[cs, :]).astype(bf)
        in_maps.append(
            {"xt": xt, "wqk": wqk, "wv": wv, "wo": wo, "mtril": mtril,
             "ident": ident})
    return in_maps


def kernel(x, w_qkv, b_qkv, w_out, b_out):
    from concourse.bass_utils import run_bass_kernel_spmd

    x = np.asarray(x, dtype=np.float32)
    w_qkv = np.asarray(w_qkv, dtype=np.float32)
    w_out = np.asarray(w_out, dtype=np.float32)

    nc = _get_program()
    in_maps = _make_in_maps(x, w_qkv, w_out)
    res = run_bass_kernel_spmd(nc, in_maps, list(range(NCORES))).results

    # unshard: sum the 4 TP partial z^T contributions per batch, transpose
    out = np.empty((B, S, D), dtype=np.float32)
    for b in range(B):
        acc = np.zeros((D, S), dtype=np.float32)
        for g in range(4):
            acc += res[4 * b + g]["ztp"].astype(np.float32)
        out[b] = acc.T
    return out
